# revision 3
# baseline (speedup 1.0000x reference)
"""Trainium2 Bass kernel for JonbertaSelfAttention (B=4,S=1024,DM=1024,H=16,D=64,SE=512,DF=512).

Sharding: 8 cores = (batch b = c//2) x (query-half qh = c%2). No collectives.
Layout strategy: transposed scores S^T[r_part, l_free]; softmax sums via a
ones-column appended to V in the PV matmul; relative-position bias terms
computed as banded matmuls against the (flipped) distance-embedding table and
diagonal-extracted via a DRAM round-trip with per-partition-skewed access
patterns; the query-side bias is gathered l-major and folded into the score
accumulation with PE transposes.
"""
import numpy as np
import ml_dtypes

BF16 = ml_dtypes.bfloat16
B, S, DM, H, D, SE, DF, MAXP = 4, 1024, 1024, 16, 64, 512, 512, 1024
L = 512          # query rows per core
NRT = S // 128   # 8 r-tiles
NLT = L // 128   # 4 l-tiles
NET = SE // 128  # 4 encoder r-tiles
LN_EPS = 1e-12

_CACHE = {}


def _build():
    import concourse.bass as bass
    import concourse.mybir as mybir
    import concourse.tile as tile
    from concourse import bacc
    from concourse.masks import make_identity
    from contextlib import ExitStack

    dt = mybir.dt
    nc = bacc.Bacc("TRN2", target_bir_lowering=False, debug=False, num_devices=8)

    # ---- DRAM I/O ----
    d_hsT = nc.dram_tensor("hsT", [DM, S], dt.bfloat16, kind="ExternalInput")
    d_hsres = nc.dram_tensor("hsres", [L, DM], dt.float32, kind="ExternalInput")
    d_encT = nc.dram_tensor("encT", [DF, SE], dt.bfloat16, kind="ExternalInput")
    d_mask = nc.dram_tensor("mask", [S], dt.float32, kind="ExternalInput")
    d_wqT = nc.dram_tensor("wqT", [DM, DM], dt.bfloat16, kind="ExternalInput")
    d_wkT = nc.dram_tensor("wkT", [DM, DM], dt.bfloat16, kind="ExternalInput")
    d_wvT = nc.dram_tensor("wvT", [DM, DM], dt.bfloat16, kind="ExternalInput")
    d_wfkT = nc.dram_tensor("wfkT", [DF, DM], dt.bfloat16, kind="ExternalInput")
    d_wfvT = nc.dram_tensor("wfvT", [DF, DM], dt.bfloat16, kind="ExternalInput")
    d_woT = nc.dram_tensor("woT", [DM, DM], dt.bfloat16, kind="ExternalInput")
    d_bq = nc.dram_tensor("bq", [DM], dt.float32, kind="ExternalInput")
    d_bk = nc.dram_tensor("bk", [DM], dt.float32, kind="ExternalInput")
    d_bfk = nc.dram_tensor("bfk", [DM], dt.float32, kind="ExternalInput")
    d_bv = nc.dram_tensor("bv", [DM], dt.bfloat16, kind="ExternalInput")
    d_bfv = nc.dram_tensor("bfv", [DM], dt.bfloat16, kind="ExternalInput")
    d_bo = nc.dram_tensor("bo", [DM], dt.float32, kind="ExternalInput")
    d_lng = nc.dram_tensor("lng", [DM], dt.float32, kind="ExternalInput")
    d_lnb = nc.dram_tensor("lnb", [DM], dt.float32, kind="ExternalInput")
    d_distn = nc.dram_tensor("distn", [2048, D], dt.bfloat16, kind="ExternalInput")
    d_distf = nc.dram_tensor("distf", [2048, D], dt.bfloat16, kind="ExternalInput")
    d_l0 = nc.dram_tensor("l0v", [1], dt.float32, kind="ExternalInput")  # unused on device
    d_out = nc.dram_tensor("out", [L, DM], dt.float32, kind="ExternalOutput")

    AP = bass.AP
    f32 = dt.float32
    bf16 = dt.bfloat16
    AF = mybir.ActivationFunctionType

    with tile.TileContext(nc) as tc, ExitStack() as top:
        # ---------- persistent pools ----------
        scr = top.enter_context(tc.tile_pool(name="scr", bufs=H * NLT, space="DRAM"))
        scr2 = top.enter_context(tc.tile_pool(name="scr2", bufs=H * NRT, space="DRAM"))
        scr3 = top.enter_context(tc.tile_pool(name="scr3", bufs=4, space="DRAM"))
        pers = top.enter_context(tc.tile_pool(name="pers", bufs=1))
        kT = pers.tile([128, NRT, S], bf16, tag="kT")
        qT = pers.tile([128, NRT, L], bf16, tag="qT")
        fkT = pers.tile([128, NRT, SE], bf16, tag="fkT")
        v_sb = pers.tile([128, NRT, H, 65], bf16, tag="v_sb")
        fv_sb = pers.tile([128, NET, H, 65], bf16, tag="fv_sb")
        hsres = pers.tile([128, NLT, DM], f32, tag="hsres")
        ctxpk = pers.tile([128, NRT, L], bf16, tag="ctxpk")
        bv_b = pers.tile([128, DM], bf16, tag="bv_b")
        bfv_b = pers.tile([128, DM], bf16, tag="bfv_b")
        lng_b = pers.tile([128, DM], f32, tag="lng_b")
        lnb_b = pers.tile([128, DM], f32, tag="lnb_b")
        bo_b = pers.tile([128, DM], f32, tag="bo_b")
        bq_s = pers.tile([128, NRT], f32, tag="bq_s")
        bk_s = pers.tile([128, NRT], f32, tag="bk_s")
        bfk_s = pers.tile([128, NRT], f32, tag="bfk_s")
        msk = pers.tile([128, NRT], f32, tag="msk")
        ident = pers.tile([128, 128], bf16, tag="ident")
        eps_t = pers.tile([128, 1], f32, tag="eps_t")
        zero_t = pers.tile([128, 1], f32, tag="zero_t")

        make_identity(nc, ident[:])
        nc.vector.memset(eps_t[:], LN_EPS)
        nc.vector.memset(zero_t[:], 0.0)
        nc.sync.dma_start(out=bv_b[:], in_=AP(tensor=d_bv, offset=0, ap=[[0, 128], [1, DM]]))
        nc.sync.dma_start(out=bfv_b[:], in_=AP(tensor=d_bfv, offset=0, ap=[[0, 128], [1, DM]]))
        nc.sync.dma_start(out=lng_b[:], in_=AP(tensor=d_lng, offset=0, ap=[[0, 128], [1, DM]]))
        nc.sync.dma_start(out=lnb_b[:], in_=AP(tensor=d_lnb, offset=0, ap=[[0, 128], [1, DM]]))
        nc.sync.dma_start(out=bo_b[:], in_=AP(tensor=d_bo, offset=0, ap=[[0, 128], [1, DM]]))
        nc.sync.dma_start(out=bq_s[:], in_=AP(tensor=d_bq, offset=0, ap=[[1, 128], [128, NRT]]))
        nc.sync.dma_start(out=bk_s[:], in_=AP(tensor=d_bk, offset=0, ap=[[1, 128], [128, NRT]]))
        nc.sync.dma_start(out=bfk_s[:], in_=AP(tensor=d_bfk, offset=0, ap=[[1, 128], [128, NRT]]))
        nc.sync.dma_start(out=msk[:], in_=AP(tensor=d_mask, offset=0, ap=[[1, 128], [128, NRT]]))
        for st in range(NLT):
            nc.sync.dma_start(out=hsres[:, st, :], in_=d_hsres[st * 128:(st + 1) * 128, :])
            nc.vector.tensor_add(out=hsres[:, st, :], in0=hsres[:, st, :], in1=bo_b[:])
        # ones column in v/fv
        nc.vector.memset(v_sb[:, :, :, 64:65], 1.0)
        nc.vector.memset(fv_sb[:, :, :, 64:65], 1.0)

        with ExitStack() as phB:
            # ---------- phase B+C0 pools ----------
            pb = phB.enter_context(tc.tile_pool(name="pb", bufs=1))
            hsT = pb.tile([128, NRT, S], bf16, tag="hsT")
            encT = pb.tile([128, NET, SE], bf16, tag="encT")
            wv_k = pb.tile([128, NRT, DM], bf16, tag="wv_k")
            wfv_k = pb.tile([128, NET, DM], bf16, tag="wfv_k")
            distn_s = pb.tile([128, 2048], bf16, tag="distn")
            distf_s = pb.tile([128, 2048], bf16, tag="distf")
            wst = phB.enter_context(tc.tile_pool(name="wst", bufs=6))
            bsb = phB.enter_context(tc.tile_pool(name="bsb", bufs=3))
            pp_proj = phB.enter_context(tc.tile_pool(name="pp_proj", bufs=2, space="PSUM"))
            pp_band = phB.enter_context(tc.tile_pool(name="pp_band", bufs=2, space="PSUM"))

            for it in range(NRT):
                nc.sync.dma_start(out=hsT[:, it, :], in_=d_hsT[it * 128:(it + 1) * 128, :])
                nc.sync.dma_start(out=wv_k[:, it, :], in_=d_wvT[it * 128:(it + 1) * 128, :])
            for it in range(NET):
                nc.sync.dma_start(out=encT[:, it, :], in_=d_encT[it * 128:(it + 1) * 128, :])
                nc.sync.dma_start(out=wfv_k[:, it, :], in_=d_wfvT[it * 128:(it + 1) * 128, :])
            # distance tables duplicated across both partition halves
            for half in range(2):
                nc.sync.dma_start(out=distn_s[half * 64:(half + 1) * 64, :],
                                  in_=AP(tensor=d_distn, offset=0, ap=[[1, 64], [64, 2048]]))
                nc.sync.dma_start(out=distf_s[half * 64:(half + 1) * 64, :],
                                  in_=AP(tensor=d_distf, offset=0, ap=[[1, 64], [64, 2048]]))

            cq_dram = {}
            ck_dram = {}

            def emit_bands(h, l0):
                hp = (h % 2) * 64
                ot = h // 2
                for lt in range(NLT):
                    bm = 896 - (l0 + lt * 128)
                    ps = pp_band.tile([128, 1152], f32, tag="band")
                    for i, (n0, nn) in enumerate(((0, 512), (512, 512), (1024, 128))):
                        nc.tensor.matmul(ps[:, n0:n0 + nn],
                                         lhsT=qT[hp:hp + 64, ot, lt * 128:(lt + 1) * 128],
                                         rhs=distf_s[hp:hp + 64, bm + n0:bm + n0 + nn],
                                         start=True, stop=True)
                    sb = bsb.tile([128, 1152], bf16, tag="band_sb")
                    nc.scalar.copy(out=sb[:], in_=ps[:])
                    dtile = scr.tile([128, 1152], bf16, tag="cq")
                    cq_dram[(h, lt)] = dtile
                    nc.sync.dma_start(out=dtile[:], in_=sb[:])
                for rt in range(NRT):
                    bt = l0 + 896 - 128 * rt
                    ps = pp_band.tile([128, 1152], f32, tag="band")
                    for i, (n0, nn) in enumerate(((0, 512), (512, 128))):
                        nc.tensor.matmul(ps[:, n0:n0 + nn],
                                         lhsT=kT[hp:hp + 64, ot, rt * 128:(rt + 1) * 128],
                                         rhs=distn_s[hp:hp + 64, bt + n0:bt + n0 + nn],
                                         start=True, stop=True)
                    sb = bsb.tile([128, 1152], bf16, tag="band_sb")
                    nc.scalar.copy(out=sb[:, 0:640], in_=ps[:, 0:640])
                    dtile = scr2.tile([128, 640], bf16, tag="ck")
                    ck_dram[(h, rt)] = dtile
                    nc.sync.dma_start(out=dtile[:], in_=sb[:, 0:640])

            # projections interleaved with band generation (l0 handled via input data:
            # qT is already the local half since host sends hsT full; we slice columns)
            # NOTE: l0 is needed for band offsets -> must be compile-time. We compile
            # one NEFF per qh variant instead (see _build_for below). Placeholder here.
            raise RuntimeError("use _build_for")

    return nc


def _build_for(qh):
    import concourse.bass as bass
    import concourse.mybir as mybir
    import concourse.tile as tile
    from concourse import bacc
    from concourse.masks import make_identity
    from contextlib import ExitStack

    l0 = qh * L
    dt = mybir.dt
    nc = bacc.Bacc("TRN2", target_bir_lowering=False, debug=False, num_devices=8)

    d_hsT = nc.dram_tensor("hsT", [DM, S], dt.bfloat16, kind="ExternalInput")
    d_hsres = nc.dram_tensor("hsres", [L, DM], dt.float32, kind="ExternalInput")
    d_encT = nc.dram_tensor("encT", [DF, SE], dt.bfloat16, kind="ExternalInput")
    d_mask = nc.dram_tensor("mask", [S], dt.float32, kind="ExternalInput")
    d_wqT = nc.dram_tensor("wqT", [DM, DM], dt.bfloat16, kind="ExternalInput")
    d_wkT = nc.dram_tensor("wkT", [DM, DM], dt.bfloat16, kind="ExternalInput")
    d_wvT = nc.dram_tensor("wvT", [DM, DM], dt.bfloat16, kind="ExternalInput")
    d_wfkT = nc.dram_tensor("wfkT", [DF, DM], dt.bfloat16, kind="ExternalInput")
    d_wfvT = nc.dram_tensor("wfvT", [DF, DM], dt.bfloat16, kind="ExternalInput")
    d_woT = nc.dram_tensor("woT", [DM, DM], dt.bfloat16, kind="ExternalInput")
    d_bq = nc.dram_tensor("bq", [DM], dt.float32, kind="ExternalInput")
    d_bk = nc.dram_tensor("bk", [DM], dt.float32, kind="ExternalInput")
    d_bfk = nc.dram_tensor("bfk", [DM], dt.float32, kind="ExternalInput")
    d_bv = nc.dram_tensor("bv", [DM], dt.bfloat16, kind="ExternalInput")
    d_bfv = nc.dram_tensor("bfv", [DM], dt.bfloat16, kind="ExternalInput")
    d_bo = nc.dram_tensor("bo", [DM], dt.float32, kind="ExternalInput")
    d_lng = nc.dram_tensor("lng", [DM], dt.float32, kind="ExternalInput")
    d_lnb = nc.dram_tensor("lnb", [DM], dt.float32, kind="ExternalInput")
    d_distn = nc.dram_tensor("distn", [2048, D], dt.bfloat16, kind="ExternalInput")
    d_distf = nc.dram_tensor("distf", [2048, D], dt.bfloat16, kind="ExternalInput")
    d_out = nc.dram_tensor("out", [L, DM], dt.float32, kind="ExternalOutput")

    AP = bass.AP
    f32 = dt.float32
    bf16 = dt.bfloat16
    AF = mybir.ActivationFunctionType

    with tile.TileContext(nc) as tc, ExitStack() as top:
        scr = top.enter_context(tc.tile_pool(name="scr", bufs=H * NLT, space="DRAM"))
        scr2 = top.enter_context(tc.tile_pool(name="scr2", bufs=H * NRT, space="DRAM"))
        scr3 = top.enter_context(tc.tile_pool(name="scr3", bufs=6, space="DRAM"))
        pers = top.enter_context(tc.tile_pool(name="pers", bufs=1))
        kT = pers.tile([128, NRT, S], bf16, tag="kT")
        qT = pers.tile([128, NRT, L], bf16, tag="qT")
        fkT = pers.tile([128, NRT, SE], bf16, tag="fkT")
        v_sb = pers.tile([128, NRT, H, 65], bf16, tag="v_sb")
        fv_sb = pers.tile([128, NET, H, 65], bf16, tag="fv_sb")
        hsres = pers.tile([128, NLT, DM], f32, tag="hsres")
        ctxpk = pers.tile([128, NRT, L], bf16, tag="ctxpk")
        bv_b = pers.tile([128, DM], bf16, tag="bv_b")
        bfv_b = pers.tile([128, DM], bf16, tag="bfv_b")
        lng_b = pers.tile([128, DM], f32, tag="lng_b")
        lnb_b = pers.tile([128, DM], f32, tag="lnb_b")
        bo_b = pers.tile([128, DM], f32, tag="bo_b")
        bq_s = pers.tile([128, NRT], f32, tag="bq_s")
        bk_s = pers.tile([128, NRT], f32, tag="bk_s")
        bfk_s = pers.tile([128, NRT], f32, tag="bfk_s")
        msk = pers.tile([128, NRT], f32, tag="msk")
        ident = pers.tile([128, 128], bf16, tag="ident")
        eps_t = pers.tile([128, 1], f32, tag="eps_t")
        zero_t = pers.tile([128, 1], f32, tag="zero_t")

        make_identity(nc, ident[:])
        nc.vector.memset(eps_t[:], LN_EPS)
        nc.vector.memset(zero_t[:], 0.0)
        nc.sync.dma_start(out=bv_b[:], in_=AP(tensor=d_bv, offset=0, ap=[[0, 128], [1, DM]]))
        nc.sync.dma_start(out=bfv_b[:], in_=AP(tensor=d_bfv, offset=0, ap=[[0, 128], [1, DM]]))
        nc.sync.dma_start(out=lng_b[:], in_=AP(tensor=d_lng, offset=0, ap=[[0, 128], [1, DM]]))
        nc.sync.dma_start(out=lnb_b[:], in_=AP(tensor=d_lnb, offset=0, ap=[[0, 128], [1, DM]]))
        nc.sync.dma_start(out=bo_b[:], in_=AP(tensor=d_bo, offset=0, ap=[[0, 128], [1, DM]]))
        nc.sync.dma_start(out=bq_s[:], in_=AP(tensor=d_bq, offset=0, ap=[[1, 128], [128, NRT]]))
        nc.sync.dma_start(out=bk_s[:], in_=AP(tensor=d_bk, offset=0, ap=[[1, 128], [128, NRT]]))
        nc.sync.dma_start(out=bfk_s[:], in_=AP(tensor=d_bfk, offset=0, ap=[[1, 128], [128, NRT]]))
        nc.sync.dma_start(out=msk[:], in_=AP(tensor=d_mask, offset=0, ap=[[1, 128], [128, NRT]]))
        for st in range(NLT):
            nc.sync.dma_start(out=hsres[:, st, :], in_=d_hsres[st * 128:(st + 1) * 128, :])
            nc.vector.tensor_add(out=hsres[:, st, :], in0=hsres[:, st, :], in1=bo_b[:])
        nc.vector.memset(v_sb[:, :, :, 64:65], 1.0)
        nc.vector.memset(fv_sb[:, :, :, 64:65], 1.0)

        cq_dram = {}
        ck_dram = {}

        with ExitStack() as phB:
            pb = phB.enter_context(tc.tile_pool(name="pb", bufs=1))
            hsT = pb.tile([128, NRT, S], bf16, tag="hsT")
            encT = pb.tile([128, NET, SE], bf16, tag="encT")
            wv_k = pb.tile([128, NRT, DM], bf16, tag="wv_k")
            wfv_k = pb.tile([128, NET, DM], bf16, tag="wfv_k")
            distn_s = pb.tile([128, 2048], bf16, tag="distn")
            distf_s = pb.tile([128, 2048], bf16, tag="distf")
            wst = phB.enter_context(tc.tile_pool(name="wst", bufs=6))
            bsb = phB.enter_context(tc.tile_pool(name="bsb", bufs=3))
            pp_proj = phB.enter_context(tc.tile_pool(name="pp_proj", bufs=2, space="PSUM"))
            pp_band = phB.enter_context(tc.tile_pool(name="pp_band", bufs=2, space="PSUM"))

            for it in range(NRT):
                nc.sync.dma_start(out=hsT[:, it, :], in_=d_hsT[it * 128:(it + 1) * 128, :])
                nc.sync.dma_start(out=wv_k[:, it, :], in_=d_wvT[it * 128:(it + 1) * 128, :])
            for it in range(NET):
                nc.sync.dma_start(out=encT[:, it, :], in_=d_encT[it * 128:(it + 1) * 128, :])
                nc.sync.dma_start(out=wfv_k[:, it, :], in_=d_wfvT[it * 128:(it + 1) * 128, :])
            for half in range(2):
                nc.sync.dma_start(out=distn_s[half * 64:(half + 1) * 64, :],
                                  in_=AP(tensor=d_distn, offset=0, ap=[[1, 64], [64, 2048]]))
                nc.sync.dma_start(out=distf_s[half * 64:(half + 1) * 64, :],
                                  in_=AP(tensor=d_distf, offset=0, ap=[[1, 64], [64, 2048]]))

            def emit_bands(h):
                hp = (h % 2) * 64
                ot = h // 2
                for lt in range(NLT):
                    bm = 896 - (l0 + lt * 128)
                    ps = pp_band.tile([128, 1152], f32, tag="band")
                    for n0, nn in ((0, 512), (512, 512), (1024, 128)):
                        nc.tensor.matmul(ps[:, n0:n0 + nn],
                                         lhsT=qT[hp:hp + 64, ot, lt * 128:(lt + 1) * 128],
                                         rhs=distf_s[hp:hp + 64, bm + n0:bm + n0 + nn],
                                         start=True, stop=True)
                    sb = bsb.tile([128, 1152], bf16, tag="band_sb")
                    nc.scalar.copy(out=sb[:], in_=ps[:])
                    dtile = scr.tile([128, 1152], bf16, tag="cq")
                    cq_dram[(h, lt)] = dtile
                    nc.sync.dma_start(out=dtile[:], in_=sb[:])
                for rt in range(NRT):
                    bt = l0 + 896 - 128 * rt
                    ps = pp_band.tile([128, 1152], f32, tag="band")
                    for n0, nn in ((0, 512), (512, 128)):
                        nc.tensor.matmul(ps[:, n0:n0 + nn],
                                         lhsT=kT[hp:hp + 64, ot, rt * 128:(rt + 1) * 128],
                                         rhs=distn_s[hp:hp + 64, bt + n0:bt + n0 + nn],
                                         start=True, stop=True)
                    sb = bsb.tile([128, 1152], bf16, tag="band_sb")
                    nc.scalar.copy(out=sb[:, 0:640], in_=ps[:, 0:640])
                    dtile = scr2.tile([128, 640], bf16, tag="ck")
                    ck_dram[(h, rt)] = dtile
                    nc.sync.dma_start(out=dtile[:], in_=sb[:, 0:640])

            for ot in range(NRT):
                # qT o-tile (local half of queries)
                ps = pp_proj.tile([128, 512], f32, tag="proj")
                for it in range(NRT):
                    w = wst.tile([128, 128], bf16, tag="wblk")
                    nc.sync.dma_start(out=w[:], in_=d_wqT[it * 128:(it + 1) * 128, ot * 128:(ot + 1) * 128])
                    nc.tensor.matmul(ps[:], lhsT=w[:], rhs=hsT[:, it, l0:l0 + L],
                                     start=(it == 0), stop=(it == NRT - 1))
                nc.scalar.activation(out=qT[:, ot, :], in_=ps[:], func=AF.Identity,
                                     bias=bq_s[:, ot:ot + 1], scale=1.0)
                # kT o-tile (full sequence)
                for sb_i in range(2):
                    ps = pp_proj.tile([128, 512], f32, tag="proj")
                    for it in range(NRT):
                        w = wst.tile([128, 128], bf16, tag="wblk")
                        nc.sync.dma_start(out=w[:], in_=d_wkT[it * 128:(it + 1) * 128, ot * 128:(ot + 1) * 128])
                        nc.tensor.matmul(ps[:], lhsT=w[:], rhs=hsT[:, it, sb_i * 512:(sb_i + 1) * 512],
                                         start=(it == 0), stop=(it == NRT - 1))
                    nc.scalar.activation(out=kT[:, ot, sb_i * 512:(sb_i + 1) * 512], in_=ps[:],
                                         func=AF.Identity, bias=bk_s[:, ot:ot + 1], scale=1.0)
                # fkT o-tile
                ps = pp_proj.tile([128, 512], f32, tag="proj")
                for it in range(NET):
                    w = wst.tile([128, 128], bf16, tag="wblk")
                    nc.sync.dma_start(out=w[:], in_=d_wfkT[it * 128:(it + 1) * 128, ot * 128:(ot + 1) * 128])
                    nc.tensor.matmul(ps[:], lhsT=w[:], rhs=encT[:, it, :],
                                     start=(it == 0), stop=(it == NET - 1))
                nc.scalar.activation(out=fkT[:, ot, :], in_=ps[:], func=AF.Identity,
                                     bias=bfk_s[:, ot:ot + 1], scale=1.0)
                emit_bands(2 * ot)
                emit_bands(2 * ot + 1)

            # V projection (s-major) and FV
            for st in range(NRT):
                for ob in range(2):
                    ps = pp_proj.tile([128, 512], f32, tag="proj")
                    for it in range(NRT):
                        nc.tensor.matmul(ps[:], lhsT=hsT[:, it, st * 128:(st + 1) * 128],
                                         rhs=wv_k[:, it, ob * 512:(ob + 1) * 512],
                                         start=(it == 0), stop=(it == NRT - 1))
                    nc.vector.tensor_add(
                        out=v_sb[:, st, ob * 8:(ob + 1) * 8, 0:64],
                        in0=ps[:].rearrange("p (h d) -> p h d", d=64),
                        in1=bv_b[:, ob * 512:(ob + 1) * 512].rearrange("p (h d) -> p h d", d=64))
            for st in range(NET):
                for ob in range(2):
                    ps = pp_proj.tile([128, 512], f32, tag="proj")
                    for it in range(NET):
                        nc.tensor.matmul(ps[:], lhsT=encT[:, it, st * 128:(st + 1) * 128],
                                         rhs=wfv_k[:, it, ob * 512:(ob + 1) * 512],
                                         start=(it == 0), stop=(it == NET - 1))
                    nc.vector.tensor_add(
                        out=fv_sb[:, st, ob * 8:(ob + 1) * 8, 0:64],
                        in0=ps[:].rearrange("p (h d) -> p h d", d=64),
                        in1=bfv_b[:, ob * 512:(ob + 1) * 512].rearrange("p (h d) -> p h d", d=64))

        # ---------- attention phase ----------
        with ExitStack() as phC:
            gp = phC.enter_context(tc.tile_pool(name="gp", bufs=8))
            g2 = phC.enter_context(tc.tile_pool(name="g2", bufs=3))
            ep = phC.enter_context(tc.tile_pool(name="ep", bufs=4))
            sp = phC.enter_context(tc.tile_pool(name="sp", bufs=3))
            cp = phC.enter_context(tc.tile_pool(name="cp", bufs=4))
            rp = phC.enter_context(tc.tile_pool(name="rp", bufs=4))
            pp_s = phC.enter_context(tc.tile_pool(name="pp_s", bufs=2, space="PSUM"))
            pp_b1 = phC.enter_context(tc.tile_pool(name="pp_b1", bufs=2, space="PSUM"))
            pp_c = phC.enter_context(tc.tile_pool(name="pp_c", bufs=4, space="PSUM"))

            for h in range(H):
                hp = (h % 2) * 64
                ot = h // 2
                ctx_ps = pp_c.tile([65, 512], f32, tag="ctx")
                ctxe_ps = pp_c.tile([65, 512], f32, tag="ctx")
                for rt in range(NRT):
                    ps = pp_s.tile([128, 512], f32, tag="sc")
                    nc.tensor.matmul(ps[:], lhsT=kT[hp:hp + 64, ot, rt * 128:(rt + 1) * 128],
                                     rhs=qT[hp:hp + 64, ot, :], start=True, stop=True)
                    pb1 = pp_b1.tile([128, 512], bf16, tag="b1ps")
                    for lt in range(NLT):
                        b1c = gp.tile([128, 128], bf16, tag="b1c")
                        src = cq_dram[(h, lt)]
                        nc.sync.dma_start(out=b1c[:], in_=AP(
                            tensor=src.tensor, offset=src.offset + 128 * rt + 127,
                            ap=[[1151, 128], [1, 128]]))
                        nc.tensor.matmul(pb1[:, lt * 128:(lt + 1) * 128], lhsT=b1c[:], rhs=ident[:],
                                         is_transpose=True, start=True, stop=True,
                                         skip_group_check=True)
                    b2t = g2.tile([128, 512], bf16, tag="b2t")
                    src = ck_dram[(h, rt)]
                    nc.sync.dma_start(out=b2t[:], in_=AP(
                        tensor=src.tensor, offset=src.offset + 127,
                        ap=[[639, 128], [1, 512]]))
                    ssb = sp.tile([128, 512], f32, tag="ssb")
                    nc.vector.tensor_add(out=ssb[:], in0=ps[:], in1=b2t[:])
                    nc.vector.tensor_add(out=ssb[:], in0=ssb[:], in1=pb1[:])
                    ex = ep.tile([128, 512], bf16, tag="ex")
                    nc.scalar.activation(out=ex[:], in_=ssb[:], func=AF.Exp,
                                         bias=msk[:, rt:rt + 1], scale=0.125)
                    nc.tensor.matmul(ctx_ps[:], lhsT=v_sb[:, rt, h, :], rhs=ex[:],
                                     start=(rt == 0), stop=(rt == NRT - 1))
                for ret in range(NET):
                    ps = pp_s.tile([128, 512], f32, tag="sc")
                    nc.tensor.matmul(ps[:], lhsT=fkT[hp:hp + 64, ot, ret * 128:(ret + 1) * 128],
                                     rhs=qT[hp:hp + 64, ot, :], start=True, stop=True)
                    ex = ep.tile([128, 512], bf16, tag="ex")
                    nc.scalar.activation(out=ex[:], in_=ps[:], func=AF.Exp,
                                         bias=zero_t[:], scale=0.125)
                    nc.tensor.matmul(ctxe_ps[:], lhsT=fv_sb[:, ret, h, :], rhs=ex[:],
                                     start=(ret == 0), stop=(ret == NET - 1))
                # normalize + combine: bounce reciprocal rows via DRAM for broadcast
                rec = rp.tile([65, 512], f32, tag="rec")
                rece = rp.tile([65, 512], f32, tag="rec")
                nc.vector.reciprocal(out=rec[64:65, :], in_=ctx_ps[64:65, :])
                nc.vector.reciprocal(out=rece[64:65, :], in_=ctxe_ps[64:65, :])
                dr1 = scr3.tile([1, 512], f32, tag="recd")
                dr2 = scr3.tile([1, 512], f32, tag="recd")
                nc.sync.dma_start(out=dr1[:], in_=rec[64:65, :])
                nc.sync.dma_start(out=dr2[:], in_=rece[64:65, :])
                rb1 = rp.tile([64, 512], f32, tag="rb")
                rb2 = rp.tile([64, 512], f32, tag="rb")
                nc.sync.dma_start(out=rb1[:], in_=AP(tensor=dr1.tensor, offset=dr1.offset,
                                                     ap=[[0, 64], [1, 512]]))
                nc.sync.dma_start(out=rb2[:], in_=AP(tensor=dr2.tensor, offset=dr2.offset,
                                                     ap=[[0, 64], [1, 512]]))
                t1 = cp.tile([64, 512], f32, tag="t1")
                t2 = cp.tile([64, 512], f32, tag="t2")
                nc.vector.tensor_mul(out=t1[:], in0=ctx_ps[0:64, :], in1=rb1[:])
                nc.vector.tensor_mul(out=t2[:], in0=ctxe_ps[0:64, :], in1=rb2[:])
                tc_ = cp.tile([64, 512], bf16, tag="tc")
                nc.vector.tensor_add(out=tc_[:], in0=t1[:], in1=t2[:])
                nc.sync.dma_start(out=ctxpk[hp:hp + 64, ot, :], in_=tc_[:])

        # ---------- output dense + residual + LN ----------
        with ExitStack() as phD:
            pd = phD.enter_context(tc.tile_pool(name="pd", bufs=1))
            wo_sb = pd.tile([128, NRT, DM], bf16, tag="wo_sb")
            yp = phD.enter_context(tc.tile_pool(name="yp", bufs=2))
            op = phD.enter_context(tc.tile_pool(name="op", bufs=2))
            stp = phD.enter_context(tc.tile_pool(name="stp", bufs=2))
            pp_y = phD.enter_context(tc.tile_pool(name="pp_y", bufs=2, space="PSUM"))

            for it in range(NRT):
                nc.sync.dma_start(out=wo_sb[:, it, :], in_=d_woT[it * 128:(it + 1) * 128, :])
            for st in range(NLT):
                y = yp.tile([128, DM], f32, tag="y")
                for ob in range(2):
                    ps = pp_y.tile([128, 512], f32, tag="py")
                    for ct in range(NRT):
                        nc.tensor.matmul(ps[:], lhsT=ctxpk[:, ct, st * 128:(st + 1) * 128],
                                         rhs=wo_sb[:, ct, ob * 512:(ob + 1) * 512],
                                         start=(ct == 0), stop=(ct == NRT - 1))
                    nc.vector.tensor_add(out=y[:, ob * 512:(ob + 1) * 512], in0=ps[:],
                                         in1=hsres[:, st, ob * 512:(ob + 1) * 512])
                stats = stp.tile([128, 2, 6], f32, tag="stats")
                nc.vector.bn_stats(out=stats[:, 0, :], in_=y[:, 0:512])
                nc.vector.bn_stats(out=stats[:, 1, :], in_=y[:, 512:1024])
                mv = stp.tile([128, 2], f32, tag="mv")
                nc.vector.bn_aggr(out=mv[:], in_=stats[:])
                sd = stp.tile([128, 1], f32, tag="sd")
                nc.scalar.activation(out=sd[:], in_=mv[:, 1:2], func=AF.Sqrt,
                                     bias=eps_t[:], scale=1.0)
                rstd = stp.tile([128, 1], f32, tag="rstd")
                nc.vector.reciprocal(out=rstd[:], in_=sd[:])
                o1 = op.tile([128, DM], f32, tag="o1")
                nc.vector.tensor_scalar(out=o1[:], in0=y[:], scalar1=mv[:, 0:1], scalar2=rstd[:],
                                        op0=mybir.AluOpType.subtract, op1=mybir.AluOpType.mult)
                o2 = op.tile([128, DM], f32, tag="o2")
                nc.vector.tensor_mul(out=o2[:], in0=o1[:], in1=lng_b[:])
                o3 = op.tile([128, DM], f32, tag="o3")
                nc.vector.tensor_add(out=o3[:], in0=o2[:], in1=lnb_b[:])
                nc.sync.dma_start(out=d_out[st * 128:(st + 1) * 128, :], in_=o3[:])

    nc.finalize()
    return nc


def _get_nc(qh):
    if qh not in _CACHE:
        _CACHE[qh] = _build_for(qh)
    return _CACHE[qh]


LAST_EXEC_NS = None
LAST_RESULTS = []


def kernel(**inputs):
    import os
    from concourse.bass_utils import run_bass_kernel_spmd

    global LAST_EXEC_NS, LAST_RESULTS
    trace = bool(os.environ.get("KTRACE"))
    inp = {k: np.asarray(v) for k, v in inputs.items()}
    hs = inp["hidden_states"].astype(np.float32)
    mask = inp["attention_mask"].astype(np.float32)
    enc = inp["encoder_hidden_states"].astype(np.float32)
    G = inp["dist_emb"].astype(np.float32)

    def b16(x):
        return np.ascontiguousarray(x.astype(BF16))

    shared = {
        "wqT": b16(inp["Wq"].T), "wkT": b16(inp["Wk"].T), "wvT": b16(inp["Wv"].T),
        "wfkT": b16(inp["Wfk"].T), "wfvT": b16(inp["Wfv"].T), "woT": b16(inp["Wo"].T),
        "bq": inp["bq"].astype(np.float32), "bk": inp["bk"].astype(np.float32),
        "bfk": inp["bfk"].astype(np.float32), "bv": b16(inp["bv"]), "bfv": b16(inp["bfv"]),
        "bo": inp["bo"].astype(np.float32), "lng": inp["ln_g"].astype(np.float32),
        "lnb": inp["ln_b"].astype(np.float32),
    }
    distn = np.zeros((2048, D), np.float32); distn[:2047] = G
    distf = np.zeros((2048, D), np.float32); distf[:2047] = G[::-1]
    shared["distn"] = b16(distn)
    shared["distf"] = b16(distf)

    in_maps = []
    for c in range(8):
        b, qhc = c // 2, c % 2
        l0 = qhc * L
        m = dict(shared)
        m["hsT"] = b16(hs[b].T)
        m["hsres"] = np.ascontiguousarray(hs[b, l0:l0 + L, :])
        m["encT"] = b16(enc[b].T)
        m["mask"] = np.ascontiguousarray(np.broadcast_to(mask[b, 0, 0, :], (S,)))
        in_maps.append(m)

    out = np.zeros((B, S, DM), np.float32)
    # two NEFF variants (query halves); run each on its 4 cores
    LAST_RESULTS = []
    total_ns = 0
    for qh in (0, 1):
        nc = _get_nc(qh)
        cores = [c for c in range(8) if c % 2 == qh]
        res = run_bass_kernel_spmd(nc, [in_maps[c] for c in cores], core_ids=cores,
                                   trace=trace)
        LAST_RESULTS.append(res)
        if res.exec_time_ns is not None:
            total_ns += res.exec_time_ns
        for i, c in enumerate(cores):
            b = c // 2
            out[b, qh * L:(qh + 1) * L, :] = res.results[i]["out"]
    LAST_EXEC_NS = total_ns if trace else None
    return out



# revision 4
# speedup vs baseline: 3.6266x; 3.6266x over previous
"""Trainium2 Bass kernel for JonbertaSelfAttention (B=4,S=1024,DM=1024,H=16,D=64,SE=512,DF=512).

Sharding: 8 cores = (batch b = c//2) x (query-half qh = c%2). No collectives.
Single NEFF for all cores: the query-half dependence (band-table offsets) is
moved host-side by shifting the distance tables per core and passing the
query-half slice hsqT as its own input.
Layout strategy: transposed scores S^T[r_part, l_free]; softmax sums via a
ones-column appended to V in the PV matmul; relative-position bias terms
computed as banded matmuls against the (flipped) distance-embedding table and
diagonal-extracted via a DRAM round-trip with per-partition-skewed access
patterns; the query-side bias is gathered l-major and folded into the score
accumulation with PE transposes.
"""
import numpy as np
import ml_dtypes

BF16 = ml_dtypes.bfloat16
B, S, DM, H, D, SE, DF, MAXP = 4, 1024, 1024, 16, 64, 512, 512, 1024
L = 512          # query rows per core
NRT = S // 128   # 8 r-tiles
NLT = L // 128   # 4 l-tiles
NET = SE // 128  # 4 encoder r-tiles
LN_EPS = 1e-12

_CACHE = {}


def _build():
    import concourse.bass as bass
    import concourse.mybir as mybir
    import concourse.tile as tile
    from concourse import bacc
    from concourse.masks import make_identity
    from contextlib import ExitStack

    dt = mybir.dt
    nc = bacc.Bacc("TRN2", target_bir_lowering=False, debug=False, num_devices=8)

    d_hsT = nc.dram_tensor("hsT", [DM, S], dt.bfloat16, kind="ExternalInput")
    d_hsqT = nc.dram_tensor("hsqT", [DM, L], dt.bfloat16, kind="ExternalInput")
    d_hsres = nc.dram_tensor("hsres", [L, DM], dt.float32, kind="ExternalInput")
    d_encT = nc.dram_tensor("encT", [DF, SE], dt.bfloat16, kind="ExternalInput")
    d_mask = nc.dram_tensor("mask", [S], dt.float32, kind="ExternalInput")
    d_wqT = nc.dram_tensor("wqT", [DM, DM], dt.bfloat16, kind="ExternalInput")
    d_wkT = nc.dram_tensor("wkT", [DM, DM], dt.bfloat16, kind="ExternalInput")
    d_wvT = nc.dram_tensor("wvT", [DM, DM], dt.bfloat16, kind="ExternalInput")
    d_wfkT = nc.dram_tensor("wfkT", [DF, DM], dt.bfloat16, kind="ExternalInput")
    d_wfvT = nc.dram_tensor("wfvT", [DF, DM], dt.bfloat16, kind="ExternalInput")
    d_woT = nc.dram_tensor("woT", [DM, DM], dt.bfloat16, kind="ExternalInput")
    d_bq = nc.dram_tensor("bq", [DM], dt.float32, kind="ExternalInput")
    d_bk = nc.dram_tensor("bk", [DM], dt.float32, kind="ExternalInput")
    d_bfk = nc.dram_tensor("bfk", [DM], dt.float32, kind="ExternalInput")
    d_bv = nc.dram_tensor("bv", [DM], dt.bfloat16, kind="ExternalInput")
    d_bfv = nc.dram_tensor("bfv", [DM], dt.bfloat16, kind="ExternalInput")
    d_bo = nc.dram_tensor("bo", [DM], dt.float32, kind="ExternalInput")
    d_lng = nc.dram_tensor("lng", [DM], dt.float32, kind="ExternalInput")
    d_lnb = nc.dram_tensor("lnb", [DM], dt.float32, kind="ExternalInput")
    # distance tables: host-transposed [d, j], duplicated across both
    # partition halves, and pre-shifted per core's query half.
    d_distn = nc.dram_tensor("distn", [128, 2048], dt.bfloat16, kind="ExternalInput")
    d_distf = nc.dram_tensor("distf", [128, 2048], dt.bfloat16, kind="ExternalInput")
    d_out = nc.dram_tensor("out", [L, DM], dt.float32, kind="ExternalOutput")

    AP = bass.AP
    f32 = dt.float32
    bf16 = dt.bfloat16
    AF = mybir.ActivationFunctionType

    with tile.TileContext(nc) as tc, ExitStack() as top:
        scr = top.enter_context(tc.tile_pool(name="scr", bufs=H * NLT, space="DRAM"))
        scr2 = top.enter_context(tc.tile_pool(name="scr2", bufs=H * NRT, space="DRAM"))
        scr3 = top.enter_context(tc.tile_pool(name="scr3", bufs=6, space="DRAM"))
        pers = top.enter_context(tc.tile_pool(name="pers", bufs=1))
        kT = pers.tile([128, NRT, S], bf16, tag="kT")
        qT = pers.tile([128, NRT, L], bf16, tag="qT")
        fkT = pers.tile([128, NRT, SE], bf16, tag="fkT")
        v_sb = pers.tile([128, NRT, H, 65], bf16, tag="v_sb")
        fv_sb = pers.tile([128, NET, H, 65], bf16, tag="fv_sb")
        ctxpk = pers.tile([128, NRT, L], bf16, tag="ctxpk")
        bv_b = pers.tile([128, DM], bf16, tag="bv_b")
        bfv_b = pers.tile([128, DM], bf16, tag="bfv_b")
        lng_b = pers.tile([128, DM], f32, tag="lng_b")
        lnb_b = pers.tile([128, DM], f32, tag="lnb_b")
        bo_b = pers.tile([128, DM], f32, tag="bo_b")
        bq_s = pers.tile([128, NRT], f32, tag="bq_s")
        bk_s = pers.tile([128, NRT], f32, tag="bk_s")
        bfk_s = pers.tile([128, NRT], f32, tag="bfk_s")
        msk = pers.tile([128, NRT], f32, tag="msk")
        ident = pers.tile([128, 128], bf16, tag="ident")
        eps_t = pers.tile([128, 1], f32, tag="eps_t")
        zero_t = pers.tile([128, 1], f32, tag="zero_t")

        make_identity(nc, ident[:])
        nc.vector.memset(eps_t[:], LN_EPS)
        nc.vector.memset(zero_t[:], 0.0)
        nc.sync.dma_start(out=bv_b[:], in_=AP(tensor=d_bv, offset=0, ap=[[0, 128], [1, DM]]))
        nc.sync.dma_start(out=bfv_b[:], in_=AP(tensor=d_bfv, offset=0, ap=[[0, 128], [1, DM]]))
        nc.sync.dma_start(out=lng_b[:], in_=AP(tensor=d_lng, offset=0, ap=[[0, 128], [1, DM]]))
        nc.sync.dma_start(out=lnb_b[:], in_=AP(tensor=d_lnb, offset=0, ap=[[0, 128], [1, DM]]))
        nc.sync.dma_start(out=bo_b[:], in_=AP(tensor=d_bo, offset=0, ap=[[0, 128], [1, DM]]))
        nc.sync.dma_start(out=bq_s[:], in_=AP(tensor=d_bq, offset=0, ap=[[1, 128], [128, NRT]]))
        nc.sync.dma_start(out=bk_s[:], in_=AP(tensor=d_bk, offset=0, ap=[[1, 128], [128, NRT]]))
        nc.sync.dma_start(out=bfk_s[:], in_=AP(tensor=d_bfk, offset=0, ap=[[1, 128], [128, NRT]]))
        nc.sync.dma_start(out=msk[:], in_=AP(tensor=d_mask, offset=0, ap=[[1, 128], [128, NRT]]))
        nc.vector.memset(v_sb[:, :, :, 64:65], 1.0)
        nc.vector.memset(fv_sb[:, :, :, 64:65], 1.0)

        cq_dram = {}
        ck_dram = {}

        with ExitStack() as phB:
            pb = phB.enter_context(tc.tile_pool(name="pb", bufs=1))
            hsT = pb.tile([128, NRT, S], bf16, tag="hsT")
            hsqT = pb.tile([128, NRT, L], bf16, tag="hsqT")
            encT = pb.tile([128, NET, SE], bf16, tag="encT")
            wq_k = pb.tile([128, NRT, DM], bf16, tag="wq_k")
            wk_k = pb.tile([128, NRT, DM], bf16, tag="wk_k")
            wv_k = pb.tile([128, NRT, DM], bf16, tag="wv_k")
            wfk_k = pb.tile([128, NET, DM], bf16, tag="wfk_k")
            wfv_k = pb.tile([128, NET, DM], bf16, tag="wfv_k")
            distn_s = pb.tile([128, 2048], bf16, tag="distn")
            distf_s = pb.tile([128, 2048], bf16, tag="distf")
            bsb = phB.enter_context(tc.tile_pool(name="bsb", bufs=2))
            pp_proj = phB.enter_context(tc.tile_pool(name="pp_proj", bufs=2, space="PSUM"))
            pp_band = phB.enter_context(tc.tile_pool(name="pp_band", bufs=2, space="PSUM"))

            for it in range(NRT):
                nc.sync.dma_start(out=hsT[:, it, :], in_=d_hsT[it * 128:(it + 1) * 128, :])
                nc.sync.dma_start(out=hsqT[:, it, :], in_=d_hsqT[it * 128:(it + 1) * 128, :])
                nc.sync.dma_start(out=wq_k[:, it, :], in_=d_wqT[it * 128:(it + 1) * 128, :])
                nc.sync.dma_start(out=wk_k[:, it, :], in_=d_wkT[it * 128:(it + 1) * 128, :])
                nc.sync.dma_start(out=wv_k[:, it, :], in_=d_wvT[it * 128:(it + 1) * 128, :])
            for it in range(NET):
                nc.sync.dma_start(out=encT[:, it, :], in_=d_encT[it * 128:(it + 1) * 128, :])
                nc.sync.dma_start(out=wfk_k[:, it, :], in_=d_wfkT[it * 128:(it + 1) * 128, :])
                nc.sync.dma_start(out=wfv_k[:, it, :], in_=d_wfvT[it * 128:(it + 1) * 128, :])
            nc.sync.dma_start(out=distn_s[:], in_=d_distn[:, :])
            nc.sync.dma_start(out=distf_s[:], in_=d_distf[:, :])

            def emit_bands(h):
                hp = (h % 2) * 64
                ot = h // 2
                for lt in range(NLT):
                    bm = 896 - lt * 128
                    ps = pp_band.tile([128, 1152], f32, tag="band")
                    for n0, nn in ((0, 512), (512, 512), (1024, 128)):
                        nc.tensor.matmul(ps[:, n0:n0 + nn],
                                         lhsT=qT[hp:hp + 64, ot, lt * 128:(lt + 1) * 128],
                                         rhs=distf_s[hp:hp + 64, bm + n0:bm + n0 + nn],
                                         start=True, stop=True)
                    sb = bsb.tile([128, 1152], bf16, tag="band_sb")
                    nc.scalar.copy(out=sb[:], in_=ps[:])
                    dtile = scr.tile([128, 1152], bf16, tag="cq")
                    cq_dram[(h, lt)] = dtile
                    nc.sync.dma_start(out=dtile[:], in_=sb[:])
                for rt in range(NRT):
                    bt = 896 - 128 * rt
                    ps = pp_band.tile([128, 1152], f32, tag="band")
                    for n0, nn in ((0, 512), (512, 128)):
                        nc.tensor.matmul(ps[:, n0:n0 + nn],
                                         lhsT=kT[hp:hp + 64, ot, rt * 128:(rt + 1) * 128],
                                         rhs=distn_s[hp:hp + 64, bt + n0:bt + n0 + nn],
                                         start=True, stop=True)
                    sb = bsb.tile([128, 1152], bf16, tag="band_sb")
                    nc.scalar.copy(out=sb[:, 0:640], in_=ps[:, 0:640])
                    dtile = scr2.tile([128, 640], bf16, tag="ck")
                    ck_dram[(h, rt)] = dtile
                    nc.sync.dma_start(out=dtile[:], in_=sb[:, 0:640])

            for ot in range(NRT):
                # qT o-tile (local query half)
                ps = pp_proj.tile([128, 512], f32, tag="proj")
                for it in range(NRT):
                    nc.tensor.matmul(ps[:], lhsT=wq_k[:, it, ot * 128:(ot + 1) * 128],
                                     rhs=hsqT[:, it, :],
                                     start=(it == 0), stop=(it == NRT - 1))
                nc.scalar.activation(out=qT[:, ot, :], in_=ps[:], func=AF.Identity,
                                     bias=bq_s[:, ot:ot + 1], scale=1.0)
                # kT o-tile (full sequence)
                for sb_i in range(2):
                    ps = pp_proj.tile([128, 512], f32, tag="proj")
                    for it in range(NRT):
                        nc.tensor.matmul(ps[:], lhsT=wk_k[:, it, ot * 128:(ot + 1) * 128],
                                         rhs=hsT[:, it, sb_i * 512:(sb_i + 1) * 512],
                                         start=(it == 0), stop=(it == NRT - 1))
                    nc.scalar.activation(out=kT[:, ot, sb_i * 512:(sb_i + 1) * 512], in_=ps[:],
                                         func=AF.Identity, bias=bk_s[:, ot:ot + 1], scale=1.0)
                # fkT o-tile
                ps = pp_proj.tile([128, 512], f32, tag="proj")
                for it in range(NET):
                    nc.tensor.matmul(ps[:], lhsT=wfk_k[:, it, ot * 128:(ot + 1) * 128],
                                     rhs=encT[:, it, :],
                                     start=(it == 0), stop=(it == NET - 1))
                nc.scalar.activation(out=fkT[:, ot, :], in_=ps[:], func=AF.Identity,
                                     bias=bfk_s[:, ot:ot + 1], scale=1.0)
                emit_bands(2 * ot)
                emit_bands(2 * ot + 1)

            # V projection (s-major) and FV
            for st in range(NRT):
                for ob in range(2):
                    ps = pp_proj.tile([128, 512], f32, tag="proj")
                    for it in range(NRT):
                        nc.tensor.matmul(ps[:], lhsT=hsT[:, it, st * 128:(st + 1) * 128],
                                         rhs=wv_k[:, it, ob * 512:(ob + 1) * 512],
                                         start=(it == 0), stop=(it == NRT - 1))
                    nc.vector.tensor_add(
                        out=v_sb[:, st, ob * 8:(ob + 1) * 8, 0:64],
                        in0=ps[:].rearrange("p (h d) -> p h d", d=64),
                        in1=bv_b[:, ob * 512:(ob + 1) * 512].rearrange("p (h d) -> p h d", d=64))
            for st in range(NET):
                for ob in range(2):
                    ps = pp_proj.tile([128, 512], f32, tag="proj")
                    for it in range(NET):
                        nc.tensor.matmul(ps[:], lhsT=encT[:, it, st * 128:(st + 1) * 128],
                                         rhs=wfv_k[:, it, ob * 512:(ob + 1) * 512],
                                         start=(it == 0), stop=(it == NET - 1))
                    nc.vector.tensor_add(
                        out=fv_sb[:, st, ob * 8:(ob + 1) * 8, 0:64],
                        in0=ps[:].rearrange("p (h d) -> p h d", d=64),
                        in1=bfv_b[:, ob * 512:(ob + 1) * 512].rearrange("p (h d) -> p h d", d=64))

        # ---------- attention phase ----------
        with ExitStack() as phC:
            gp = phC.enter_context(tc.tile_pool(name="gp", bufs=8))
            g2 = phC.enter_context(tc.tile_pool(name="g2", bufs=3))
            ep = phC.enter_context(tc.tile_pool(name="ep", bufs=4))
            sp = phC.enter_context(tc.tile_pool(name="sp", bufs=3))
            cp = phC.enter_context(tc.tile_pool(name="cp", bufs=4))
            rp = phC.enter_context(tc.tile_pool(name="rp", bufs=4))
            pp_s = phC.enter_context(tc.tile_pool(name="pp_s", bufs=2, space="PSUM"))
            pp_b1 = phC.enter_context(tc.tile_pool(name="pp_b1", bufs=2, space="PSUM"))
            pp_c = phC.enter_context(tc.tile_pool(name="pp_c", bufs=4, space="PSUM"))

            for h in range(H):
                hp = (h % 2) * 64
                ot = h // 2
                ctx_ps = pp_c.tile([65, 512], f32, tag="ctx")
                ctxe_ps = pp_c.tile([65, 512], f32, tag="ctx")
                # wide skewed reads: one per l-tile, covering all 8 r-tiles
                b1w = []
                for lt in range(NLT):
                    t = gp.tile([128, 1024], bf16, tag="b1w")
                    src = cq_dram[(h, lt)]
                    nc.sync.dma_start(out=t[:], in_=AP(
                        tensor=src.tensor, offset=src.offset + 127,
                        ap=[[1151, 128], [1, 1024]]))
                    b1w.append(t)
                for rt in range(NRT):
                    ps = pp_s.tile([128, 512], f32, tag="sc")
                    nc.tensor.matmul(ps[:], lhsT=kT[hp:hp + 64, ot, rt * 128:(rt + 1) * 128],
                                     rhs=qT[hp:hp + 64, ot, :], start=True, stop=True)
                    pb1 = pp_b1.tile([128, 512], bf16, tag="b1ps")
                    for lt in range(NLT):
                        nc.tensor.matmul(pb1[:, lt * 128:(lt + 1) * 128],
                                         lhsT=b1w[lt][:, rt * 128:(rt + 1) * 128], rhs=ident[:],
                                         is_transpose=True, start=True, stop=True,
                                         skip_group_check=True)
                    b2t = g2.tile([128, 512], bf16, tag="b2t")
                    src = ck_dram[(h, rt)]
                    nc.sync.dma_start(out=b2t[:], in_=AP(
                        tensor=src.tensor, offset=src.offset + 127,
                        ap=[[639, 128], [1, 512]]))
                    ssb = sp.tile([128, 512], f32, tag="ssb")
                    nc.vector.tensor_add(out=ssb[:], in0=ps[:], in1=b2t[:])
                    nc.vector.tensor_add(out=ssb[:], in0=ssb[:], in1=pb1[:])
                    ex = ep.tile([128, 512], bf16, tag="ex")
                    nc.scalar.activation(out=ex[:], in_=ssb[:], func=AF.Exp,
                                         bias=msk[:, rt:rt + 1], scale=0.125)
                    nc.tensor.matmul(ctx_ps[:], lhsT=v_sb[:, rt, h, :], rhs=ex[:],
                                     start=(rt == 0), stop=(rt == NRT - 1))
                for ret in range(NET):
                    ps = pp_s.tile([128, 512], f32, tag="sc")
                    nc.tensor.matmul(ps[:], lhsT=fkT[hp:hp + 64, ot, ret * 128:(ret + 1) * 128],
                                     rhs=qT[hp:hp + 64, ot, :], start=True, stop=True)
                    ex = ep.tile([128, 512], bf16, tag="ex")
                    nc.scalar.activation(out=ex[:], in_=ps[:], func=AF.Exp,
                                         bias=zero_t[:], scale=0.125)
                    nc.tensor.matmul(ctxe_ps[:], lhsT=fv_sb[:, ret, h, :], rhs=ex[:],
                                     start=(ret == 0), stop=(ret == NET - 1))
                # normalize + combine: bounce reciprocal rows via DRAM for broadcast
                rec = rp.tile([65, 512], f32, tag="rec")
                rece = rp.tile([65, 512], f32, tag="rec")
                nc.vector.reciprocal(out=rec[64:65, :], in_=ctx_ps[64:65, :])
                nc.vector.reciprocal(out=rece[64:65, :], in_=ctxe_ps[64:65, :])
                dr1 = scr3.tile([1, 512], f32, tag="recd")
                dr2 = scr3.tile([1, 512], f32, tag="recd")
                nc.sync.dma_start(out=dr1[:], in_=rec[64:65, :])
                nc.sync.dma_start(out=dr2[:], in_=rece[64:65, :])
                rb1 = rp.tile([64, 512], f32, tag="rb")
                rb2 = rp.tile([64, 512], f32, tag="rb")
                nc.sync.dma_start(out=rb1[:], in_=AP(tensor=dr1.tensor, offset=dr1.offset,
                                                     ap=[[0, 64], [1, 512]]))
                nc.sync.dma_start(out=rb2[:], in_=AP(tensor=dr2.tensor, offset=dr2.offset,
                                                     ap=[[0, 64], [1, 512]]))
                t1 = cp.tile([64, 512], f32, tag="t1")
                t2 = cp.tile([64, 512], f32, tag="t2")
                nc.vector.tensor_mul(out=t1[:], in0=ctx_ps[0:64, :], in1=rb1[:])
                nc.vector.tensor_mul(out=t2[:], in0=ctxe_ps[0:64, :], in1=rb2[:])
                tc_ = cp.tile([64, 512], bf16, tag="tc")
                nc.vector.tensor_add(out=tc_[:], in0=t1[:], in1=t2[:])
                nc.sync.dma_start(out=ctxpk[hp:hp + 64, ot, :], in_=tc_[:])

        # ---------- output dense + residual + LN ----------
        with ExitStack() as phD:
            pd = phD.enter_context(tc.tile_pool(name="pd", bufs=1))
            wo_sb = pd.tile([128, NRT, DM], bf16, tag="wo_sb")
            hsres = pd.tile([128, NLT, DM], f32, tag="hsres")
            yp = phD.enter_context(tc.tile_pool(name="yp", bufs=2))
            op = phD.enter_context(tc.tile_pool(name="op", bufs=2))
            stp = phD.enter_context(tc.tile_pool(name="stp", bufs=2))
            pp_y = phD.enter_context(tc.tile_pool(name="pp_y", bufs=2, space="PSUM"))

            for it in range(NRT):
                nc.sync.dma_start(out=wo_sb[:, it, :], in_=d_woT[it * 128:(it + 1) * 128, :])
            for st in range(NLT):
                nc.sync.dma_start(out=hsres[:, st, :], in_=d_hsres[st * 128:(st + 1) * 128, :])
                nc.vector.tensor_add(out=hsres[:, st, :], in0=hsres[:, st, :], in1=bo_b[:])
            for st in range(NLT):
                y = yp.tile([128, DM], f32, tag="y")
                for ob in range(2):
                    ps = pp_y.tile([128, 512], f32, tag="py")
                    for ct in range(NRT):
                        nc.tensor.matmul(ps[:], lhsT=ctxpk[:, ct, st * 128:(st + 1) * 128],
                                         rhs=wo_sb[:, ct, ob * 512:(ob + 1) * 512],
                                         start=(ct == 0), stop=(ct == NRT - 1))
                    nc.vector.tensor_add(out=y[:, ob * 512:(ob + 1) * 512], in0=ps[:],
                                         in1=hsres[:, st, ob * 512:(ob + 1) * 512])
                stats = stp.tile([128, 2, 6], f32, tag="stats")
                nc.vector.bn_stats(out=stats[:, 0, :], in_=y[:, 0:512])
                nc.vector.bn_stats(out=stats[:, 1, :], in_=y[:, 512:1024])
                mv = stp.tile([128, 2], f32, tag="mv")
                nc.vector.bn_aggr(out=mv[:], in_=stats[:])
                sd = stp.tile([128, 1], f32, tag="sd")
                nc.scalar.activation(out=sd[:], in_=mv[:, 1:2], func=AF.Sqrt,
                                     bias=eps_t[:], scale=1.0)
                rstd = stp.tile([128, 1], f32, tag="rstd")
                nc.vector.reciprocal(out=rstd[:], in_=sd[:])
                o1 = op.tile([128, DM], f32, tag="o1")
                nc.vector.tensor_scalar(out=o1[:], in0=y[:], scalar1=mv[:, 0:1], scalar2=rstd[:],
                                        op0=mybir.AluOpType.subtract, op1=mybir.AluOpType.mult)
                o2 = op.tile([128, DM], f32, tag="o2")
                nc.vector.tensor_mul(out=o2[:], in0=o1[:], in1=lng_b[:])
                o3 = op.tile([128, DM], f32, tag="o3")
                nc.vector.tensor_add(out=o3[:], in0=o2[:], in1=lnb_b[:])
                nc.sync.dma_start(out=d_out[st * 128:(st + 1) * 128, :], in_=o3[:])

    nc.finalize()
    return nc


def _get_nc():
    if "nc" not in _CACHE:
        _CACHE["nc"] = _build()
    return _CACHE["nc"]


LAST_EXEC_NS = None
LAST_RESULTS = []


def kernel(**inputs):
    import os
    from concourse.bass_utils import run_bass_kernel_spmd

    global LAST_EXEC_NS, LAST_RESULTS
    trace = bool(os.environ.get("KTRACE"))
    inp = {k: np.asarray(v) for k, v in inputs.items()}
    hs = inp["hidden_states"].astype(np.float32)
    mask = inp["attention_mask"].astype(np.float32)
    enc = inp["encoder_hidden_states"].astype(np.float32)
    G = inp["dist_emb"].astype(np.float32)

    def b16(x):
        return np.ascontiguousarray(x.astype(BF16))

    shared = {
        "wqT": b16(inp["Wq"].T), "wkT": b16(inp["Wk"].T), "wvT": b16(inp["Wv"].T),
        "wfkT": b16(inp["Wfk"].T), "wfvT": b16(inp["Wfv"].T), "woT": b16(inp["Wo"].T),
        "bq": inp["bq"].astype(np.float32), "bk": inp["bk"].astype(np.float32),
        "bfk": inp["bfk"].astype(np.float32), "bv": b16(inp["bv"]), "bfv": b16(inp["bfv"]),
        "bo": inp["bo"].astype(np.float32), "lng": inp["ln_g"].astype(np.float32),
        "lnb": inp["ln_b"].astype(np.float32),
    }
    # distance tables: [2048, 64] padded; distf flipped. Per query half qh the
    # device always indexes with qh=0 offsets, so pre-shift:
    #   distf'[x] = distf[x - 512*qh]   distn'[x] = distn[x + 512*qh]
    # then transpose to [64, 2048] and duplicate across both partition halves.
    base_n = np.zeros((2048, D), np.float32); base_n[:2047] = G
    base_f = np.zeros((2048, D), np.float32); base_f[:2047] = G[::-1]

    def pack(t):
        return b16(np.vstack([t.T, t.T]))

    dist_by_qh = {}
    for qh in (0, 1):
        if qh == 0:
            dn, dfv = base_n, base_f
        else:
            dn = np.zeros((2048, D), np.float32); dn[:1536] = base_n[512:]
            dfv = np.zeros((2048, D), np.float32); dfv[512:] = base_f[:1536]
        dist_by_qh[qh] = (pack(dn), pack(dfv))

    in_maps = []
    for c in range(8):
        b, qh = c // 2, c % 2
        l0 = qh * L
        m = dict(shared)
        m["hsT"] = b16(hs[b].T)
        m["hsqT"] = b16(hs[b, l0:l0 + L, :].T)
        m["hsres"] = np.ascontiguousarray(hs[b, l0:l0 + L, :])
        m["encT"] = b16(enc[b].T)
        m["mask"] = np.ascontiguousarray(np.broadcast_to(mask[b, 0, 0, :], (S,)))
        m["distn"], m["distf"] = dist_by_qh[qh]
        in_maps.append(m)

    nc = _get_nc()
    res = run_bass_kernel_spmd(nc, in_maps, core_ids=list(range(8)), trace=trace)
    LAST_RESULTS = [res]
    LAST_EXEC_NS = res.exec_time_ns if trace else None

    out = np.zeros((B, S, DM), np.float32)
    for c in range(8):
        b, qh = c // 2, c % 2
        out[b, qh * L:(qh + 1) * L, :] = res.results[c]["out"]
    return out
